# revision 3
# baseline (speedup 1.0000x reference)
"""Attention-LSTM decoder (nn_Decoder) Trainium2 Bass kernel, v3.

Sharding: data-parallel over batch B=64 -> 8 cores x 8 examples, with a
cross-core sort of examples by out_len: global rank r -> (core r%8,
slot r//8), so slot m has similar out_len on every core and the per-slot
attention width W_m (multiple of 128, <= 800) is baked into the program.

Per-step structure (per core):
  gates : col-tiled PE MMs as v2 (4 gate groups x 9 k-chunks, rows 32n).
          i/f/o rows of W pre-scaled by 0.5 and sigmoid computed as
          (1+tanh(x/2))/2 so the whole kernel uses only {tanh, exp,
          identity} -> zero act-table reloads. h is stored as h2=2h with
          the 0.5 folded into W_hh / W1 / the exp scale.
  lstm  : gates transposed to feature-on-partition ([128, 4ck x 8ex]) via
          4 PE transposes; pointwise STT chain on [128, 32] tiles; h2 STT
          writes hT_all directly (no h transposes, no copies).
  energy: per-slot MMs at width W_m into spread psum (slot m -> tile m//4,
          col-group m%4, psum row 32*(m%4)). psum is pre-written (DVE, off
          critical path) with a -4000 bias beyond each example's out_len,
          so exp(scale*(e2+bias)) -> 0 there: no mask multiply, no gather.
  softmax: exp ACT psum->SBUF with accum_out row-sum; reciprocal on DVE.
  ctx   : w^T via full-width PE transposes (HW forbids partition-strided
          engine APs; DVE extracts cols 0:97:32); per-slot ctx MMs;
          normalize+copy fused in one DVE tensor_scalar_mul; ctx^T the
          same way, one DVE copy per tile into ctxT_all.
  next-step gates (onehot + h chunks) issue right after h2 to keep PE fed
  through the softmax chain.
MLP head hoisted out of the loop and interleaved into the pointwise-window
PE gaps as 512-column units once their history is complete.
"""

import math
import sys
from contextlib import ExitStack

import numpy as np

sys.path.insert(0, "/opt/trn_rl_repo")

import ml_dtypes  # noqa: E402

import concourse.bass as bass  # noqa: E402
import concourse.bacc as bacc  # noqa: E402
import concourse.tile as tile  # noqa: E402
from concourse import mybir  # noqa: E402
from concourse.masks import make_identity  # noqa: E402

BF16 = ml_dtypes.bfloat16

V, E, H, C = 34, 256, 512, 512
B, T, S = 64, 200, 800
NCORES = 8
BL = B // NCORES
SCALE = 1.0 / math.sqrt(128.0)
FDT = mybir.dt.float32
BDT = mybir.dt.bfloat16
NEG_BIAS = -4000.0


def slot_width(max_len: int) -> int:
    return min(int(max_len), S)


def chunks_for(w: int):
    """[(offset, size)] 128-col chunks covering width w (last may be 32)."""
    out = []
    off = 0
    while off < w:
        out.append((off, min(128, w - off)))
        off += 128
    return out


def build_bass(t_steps: int, widths: tuple) -> bass.Bass:
    assert len(widths) == BL
    nc = bacc.Bacc()

    kts_d = nc.dram_tensor("kts", [BL, 4, 128, S], BDT, kind="ExternalInput")
    v_d = nc.dram_tensor("v", [BL, 6, 128, C], BDT, kind="ExternalInput")
    v6_d = nc.dram_tensor("v6", [128, 2, C], BDT, kind="ExternalInput")
    w_d = nc.dram_tensor("wt", [8, 128, 4 * H], BDT, kind="ExternalInput")
    ew_d = nc.dram_tensor("ewt", [V, 4, 512], BDT, kind="ExternalInput")
    oh_d = nc.dram_tensor("oh", [V, t_steps, BL], BDT, kind="ExternalInput")
    eb_d = nc.dram_tensor("ebias4", [4, 4, 512], BDT, kind="ExternalInput")
    sel_d = nc.dram_tensor("sel4", [4, 97], BDT, kind="ExternalInput")
    w1_d = nc.dram_tensor("w1t", [8, 128, C], BDT, kind="ExternalInput")
    w2_d = nc.dram_tensor("w2t", [4, 128, V], BDT, kind="ExternalInput")
    b1c_d = nc.dram_tensor("b1c", [128, 4], FDT, kind="ExternalInput")
    b2c_d = nc.dram_tensor("b2c", [V, 1], FDT, kind="ExternalInput")
    out_d = nc.dram_tensor("out", [V, t_steps * BL], BDT, kind="ExternalOutput")

    AF = mybir.ActivationFunctionType
    OP = mybir.AluOpType

    # per-tile (A: slots 0-3, B: slots 4-7) max widths
    wtA = max(widths[0:4])
    wtB = max(widths[4:8])

    with tile.TileContext(nc) as tc, ExitStack() as es:
        consts = es.enter_context(tc.tile_pool(name="consts", bufs=1))
        state = es.enter_context(tc.tile_pool(name="state", bufs=1))
        work = es.enter_context(tc.tile_pool(name="work", bufs=1))
        work2 = es.enter_context(tc.tile_pool(name="work2", bufs=2))
        psEA = es.enter_context(tc.tile_pool(name="psEA", bufs=1, space="PSUM"))
        psEB = es.enter_context(tc.tile_pool(name="psEB", bufs=1, space="PSUM"))
        psG = es.enter_context(tc.tile_pool(name="psG", bufs=1, space="PSUM"))
        psT = es.enter_context(tc.tile_pool(name="psT", bufs=1, space="PSUM"))
        psW = es.enter_context(tc.tile_pool(name="psW", bufs=1, space="PSUM"))
        psM = es.enter_context(tc.tile_pool(name="psM", bufs=1, space="PSUM"))

        # ---- resident inputs ----
        dmaq = [nc.sync, nc.scalar]
        kts_sb = consts.tile([128, BL, 4, S], BDT)
        for ex in range(BL):
            for hk in range(4):
                dmaq[(ex * 4 + hk) % 2].dma_start(
                    out=kts_sb[:, ex, hk, :], in_=kts_d[ex, hk]
                )
        v_sb = consts.tile([128, BL, 6, C], BDT)
        for ex in range(BL):
            for sj in range(6):
                dmaq[(ex * 6 + sj) % 2].dma_start(
                    out=v_sb[:, ex, sj, :], in_=v_d[ex, sj]
                )
        v6_sb = consts.tile([128, 2, C], BDT)
        nc.sync.dma_start(out=v6_sb, in_=v6_d[:, :, :])
        w_sb = consts.tile([128, 8, 4 * H], BDT)
        for k in range(8):
            dmaq[k % 2].dma_start(out=w_sb[:, k, :], in_=w_d[k])
        ew_sb = consts.tile([V, 4, 512], BDT)
        nc.sync.dma_start(out=ew_sb, in_=ew_d[:, :, :])
        oh_sb = consts.tile([V, t_steps, BL], BDT)
        nc.sync.dma_start(out=oh_sb, in_=oh_d[:, :, :])
        eb_sb = consts.tile([4, 4, 512], BDT)
        nc.sync.dma_start(out=eb_sb, in_=eb_d[:, :, :])
        sel_sb = consts.tile([4, 97], BDT)
        nc.sync.dma_start(out=sel_sb, in_=sel_d[:, :])
        w1_sb = consts.tile([128, 8, C], BDT)
        for k in range(8):
            nc.sync.dma_start(out=w1_sb[:, k, :], in_=w1_d[k])
        w2_sb = consts.tile([128, 4, V], BDT)
        for k in range(4):
            nc.sync.dma_start(out=w2_sb[:, k, :], in_=w2_d[k])
        b1c_sb = consts.tile([128, 4], FDT)
        nc.sync.dma_start(out=b1c_sb, in_=b1c_d[:, :])
        b2c_sb = consts.tile([V, 1], FDT)
        nc.sync.dma_start(out=b2c_sb, in_=b2c_d[:, :])

        id128f = consts.tile([128, 128], FDT)
        make_identity(nc, id128f)
        id128b = consts.tile([128, 128], BDT)
        nc.vector.tensor_copy(id128b, id128f)

        # ---- recurrent state ----
        NS = (t_steps + 1) * BL
        hT_all = state.tile([128, 4, NS], BDT)
        ctxT_all = state.tile([128, 4, NS], BDT)
        nc.vector.memset(hT_all[:, :, 0:BL], 0.0)
        nc.vector.memset(ctxT_all[:, :, 0:BL], 0.0)

        P_EA = psEA.tile([128, 2, 512], FDT, tag="ea")
        P_EB = psEB.tile([128, 2, 512], FDT, tag="eb")
        P_G = psG.tile([128, 512], FDT, tag="g")
        P_T = psT.tile([128, 4, 104], FDT, tag="t")
        # full-width transpose scratch (engines cannot stride partitions on
        # HW, so transposes carry 97 junk cols and DVE extracts cols 0:97:32)
        P_X = psW.tile([128, 10, 98], BDT, tag="x")
        P_WS = P_X[:, 0:2, :]
        P_CA = P_X[:, 2:6, :]
        P_CB = P_X[:, 6:10, :]
        P_M = psM.tile([128, 512], FDT, tag="m")  # MLP hid/logits psum
        nc.vector.memset(P_G, 0.0)
        nc.vector.memset(P_EA, 0.0)
        nc.vector.memset(P_EB, 0.0)

        # persistent small work tiles
        s_all = work.tile([104, 512], FDT, tag="s_all")
        gT_sb = work.tile([128, 32], FDT, tag="gT")
        A_sb = work.tile([128, 32], FDT, tag="A")
        Bp_sb = work.tile([128, 32], FDT, tag="Bp")
        tc_sb = work.tile([128, 32], BDT, tag="tc")
        wm_A = work.tile([97, 2, 512], BDT, tag="wmA")
        wm_B = work.tile([97, 2, 512], BDT, tag="wmB")
        ws = work.tile([97, 4], FDT, tag="ws")  # accums: A0 A1 B0 B1
        wsA = work.tile([97, 1], FDT, tag="wsA")
        wsB = work.tile([97, 1], FDT, tag="wsB")
        rinvA = work.tile([97, 1], FDT, tag="rinvA")
        rinvB = work.tile([97, 1], FDT, tag="rinvB")
        wT_A = work.tile([128, 7, 4], BDT, tag="wTA")
        wT_B = work.tile([128, 7, 4], BDT, tag="wTB")
        ctxs_A = work.tile([97, 512], BDT, tag="ctxsA")
        ctxs_B = work.tile([97, 512], BDT, tag="ctxsB")

        def gates1(t):
            """onehot + h k-chunks of step t's gates (needs hT slot t)."""
            for n in range(4):
                nc.tensor.matmul(
                    P_G[32 * n : 32 * n + BL, :],
                    oh_sb[:, t, :],
                    ew_sb[:, n, :],
                    start=True,
                    stop=False,
                    tile_position=(0, 32 * n),
                    skip_group_check=True,
                )
            for k in range(4):
                for n in range(4):
                    nc.tensor.matmul(
                        P_G[32 * n : 32 * n + BL, :],
                        hT_all[:, k, t * BL : (t + 1) * BL],
                        w_sb[:, 4 + k, n * 512 : (n + 1) * 512],
                        start=False,
                        stop=False,
                        tile_position=(0, 32 * n),
                        skip_group_check=True,
                    )

        def gates2_k(t, k):
            """ctx k-chunk k of step t's gates (needs ctxT[:, k, slot t])."""
            for n in range(4):
                nc.tensor.matmul(
                    P_G[32 * n : 32 * n + BL, :],
                    ctxT_all[:, k, t * BL : (t + 1) * BL],
                    w_sb[:, k, n * 512 : (n + 1) * 512],
                    start=False,
                    stop=(k == 3),
                    tile_position=(0, 32 * n),
                    skip_group_check=True,
                )

        def ebias_mm(tile_ps, tile_idx, wt):
            # write the masked -inf bias into energy psum via PE (sel4 maps
            # slot g's bias row to partition 32g, zeros elsewhere so
            # wsum/recip stay finite); PE program order makes the
            # bias -> accumulate sequencing race-free.
            nc.tensor.matmul(
                tile_ps[0:97, 0, 0:512],
                sel_sb,
                eb_sb[:, 2 * tile_idx, :],
                start=True, stop=True, skip_group_check=True,
            )
            if wt > 512:
                nc.tensor.matmul(
                    tile_ps[0:97, 1, 0 : wt - 512],
                    sel_sb,
                    eb_sb[:, 2 * tile_idx + 1, 0 : wt - 512],
                    start=True, stop=True, skip_group_check=True,
                )

        def energy(t, tile_ps, slots):
            for m in slots:
                g = m % 4
                wm = widths[m]
                col = (t + 1) * BL + m
                for hk in range(4):
                    nc.tensor.matmul(
                        tile_ps[32 * g : 32 * g + 1, 0, 0 : min(wm, 512)],
                        hT_all[:, hk, col : col + 1],
                        kts_sb[:, m, hk, 0 : min(wm, 512)],
                        start=False,
                        stop=(hk == 3),
                        tile_position=(0, 32 * g),
                        skip_group_check=True,
                    )
                    if wm > 512:
                        nc.tensor.matmul(
                            tile_ps[32 * g : 32 * g + 1, 1, 0 : wm - 512],
                            hT_all[:, hk, col : col + 1],
                            kts_sb[:, m, hk, 512:wm],
                            start=False,
                            stop=(hk == 3),
                            tile_position=(0, 32 * g),
                            skip_group_check=True,
                        )

        def exp_tile(tile_ps, wm, wt, acc0, acc1):
            w0 = min(wt, 512)
            nc.scalar.activation(
                wm[:, 0, 0:w0],
                tile_ps[0:97, 0, 0:w0],
                AF.Exp,
                scale=SCALE * 0.5,
                accum_out=acc0,
            )
            if wt > 512:
                nc.scalar.activation(
                    wm[:, 1, 0 : wt - 512],
                    tile_ps[0:97, 1, 0 : wt - 512],
                    AF.Exp,
                    scale=SCALE * 0.5,
                    accum_out=acc1,
                )

        def wT_chunks(wm, wt, wT_sb, tileslots):
            # transpose full [97, chunk] (junk cols), extract cols 0:97:32
            for sj, (off, csz) in enumerate(chunks_for(wt)):
                hi, o = (0, off) if off < 512 else (1, off - 512)
                buf = sj % 2
                nc.tensor.transpose(
                    P_WS[0:csz, buf, 0:97],
                    wm[0:97, hi, o : o + csz],
                    id128b[0:97, 0:97],
                )
                if off < 768:
                    nc.vector.tensor_copy(
                        wT_sb[0:csz, sj, :], P_WS[0:csz, buf, 0:97:32]
                    )
                else:  # s>=768 tail: slot g's rows must land at 32g..
                    for g in range(4):
                        rem = widths[tileslots[g]] - 768
                        if rem > 0:
                            nc.vector.tensor_copy(
                                wT_sb[32 * g : 32 * g + rem, sj, g : g + 1],
                                P_WS[0:rem, buf, 32 * g : 32 * g + 1],
                            )

        def ctx_mms(t, tile_ps, slots, wT_sb):
            for m in slots:
                g = m % 4
                cks = chunks_for(widths[m])
                last = len(cks) - 1
                for sj, (off, csz) in enumerate(cks):
                    if off < 768:
                        lhsT = wT_sb[0:csz, sj, g : g + 1]
                        rhs = v_sb[0:csz, m, sj, :]
                        tpos = (0, 32 * g)
                    else:  # s>=768 tail chunk (v6 packing)
                        lhsT = wT_sb[32 * g : 32 * g + csz, sj, g : g + 1]
                        rhs = v6_sb[32 * g : 32 * g + csz, m // 4, :]
                        tpos = (32 * g, 32 * g)
                    nc.tensor.matmul(
                        tile_ps[32 * g : 32 * g + 1, 0, 0:512],
                        lhsT,
                        rhs,
                        start=(sj == 0),
                        stop=(sj == last),
                        tile_position=tpos,
                        skip_group_check=True,
                    )

        # ---- MLP head units (interleaved into the t loop's PE gaps) ----
        NT = t_steps * BL
        tcs = []
        t0c = 0
        while t0c < NT:
            tcs.append((t0c, min(512, NT - t0c)))
            t0c += 512
        hidT = work.tile([128, 4, 512], BDT, tag="hidT")

        def mlp_hid(ti, cj):
            t0, tw = tcs[ti]
            for k in range(8):
                h_src = hT_all if k < 4 else ctxT_all
                nc.tensor.matmul(
                    P_M[:, 0:tw],
                    w1_sb[:, k, cj * 128 : (cj + 1) * 128],
                    h_src[:, k % 4, BL + t0 : BL + t0 + tw],
                    start=(k == 0),
                    stop=(k == 7),
                )
            nc.scalar.activation(
                hidT[:, cj, 0:tw],
                P_M[:, 0:tw],
                AF.Tanh,
                bias=b1c_sb[:, cj : cj + 1],
            )

        def mlp_logits(ti):
            t0, tw = tcs[ti]
            for ck in range(4):
                nc.tensor.matmul(
                    P_M[0:V, 0:tw],
                    w2_sb[:, ck, :],
                    hidT[:, ck, 0:tw],
                    start=(ck == 0),
                    stop=(ck == 3),
                )
            o_chunk = work2.tile([V, 512], BDT, tag="o_chunk")
            nc.scalar.activation(
                o_chunk[:, 0:tw], P_M[0:V, 0:tw], AF.Identity, bias=b2c_sb
            )
            nc.sync.dma_start(out=out_d[:, t0 : t0 + tw], in_=o_chunk[:, 0:tw])

        def emit_mlp_unit(u):
            ti, cj = u
            if cj == 4:
                mlp_logits(ti)
            else:
                mlp_hid(ti, cj)

        # unit (ti, cj) ready once steps [64*ti, 64*ti+64) are all written
        mlp_units = [
            (ti, cj) for ti in range(len(tcs)) for cj in range(5)
        ]

        def mlp_ready(u, t):
            ti, _ = u
            return t >= 64 * ti + 65 and 64 * (ti + 1) <= t_steps

        # ---- prologue: step 0's full gate accumulation ----
        gates1(0)
        for k in range(4):
            gates2_k(0, k)

        d_prev = None
        for t in range(t_steps):
            sl_out = slice((t + 1) * BL, (t + 2) * BL)

            # fill the pointwise-window PE gap: energy-psum bias writes for
            # this step (only need last step's exp to have read the banks),
            # plus one MLP head unit when available
            ebias_mm(P_EA, 0, wtA)
            ebias_mm(P_EB, 1, wtB)
            if mlp_units and mlp_ready(mlp_units[0], t):
                emit_mlp_unit(mlp_units.pop(0))

            # ---- LSTM pointwise (gates finished in previous tail) ----
            nc.scalar.activation(s_all, P_G[0:104, :], AF.Tanh)
            for ck in range(4):
                nc.tensor.transpose(
                    P_T[:, ck, :],
                    s_all[:, ck * 128 : (ck + 1) * 128],
                    id128f[0:104, 0:104],
                )
            iT = P_T[:, :, 0:8]
            fT = P_T[:, :, 32:40]
            oT = P_T[:, :, 64:72]
            # DVE can read at most one PSUM operand per op: stage gT in SBUF
            # (on the otherwise-idle ACT engine).
            nc.scalar.activation(gT_sb, P_T[:, :, 96:104], AF.Identity)
            nc.vector.scalar_tensor_tensor(
                out=A_sb, in0=iT, scalar=1.0, in1=gT_sb, op0=OP.add, op1=OP.mult
            )
            d_new = work2.tile([128, 32], FDT, tag="d")
            if d_prev is None:
                # d = 2c = (1+s_i)*g
                nc.vector.tensor_copy(d_new, A_sb)
            else:
                nc.vector.scalar_tensor_tensor(
                    out=Bp_sb, in0=fT, scalar=1.0, in1=d_prev,
                    op0=OP.add, op1=OP.mult,
                )
                nc.vector.scalar_tensor_tensor(
                    out=d_new, in0=Bp_sb, scalar=0.5, in1=A_sb,
                    op0=OP.mult, op1=OP.add,
                )
            d_prev = d_new
            nc.scalar.activation(tc_sb, d_new, AF.Tanh, scale=0.5)
            nc.vector.scalar_tensor_tensor(
                out=hT_all[:, :, sl_out], in0=oT, scalar=1.0, in1=tc_sb,
                op0=OP.add, op1=OP.mult,
            )

            # ---- attention energy (per-slot widths) ----
            energy(t, P_EA, range(0, 4))
            energy(t, P_EB, range(4, 8))
            exp_tile(P_EA, wm_A, wtA, ws[:, 0:1], ws[:, 1:2])

            # next step's onehot+h gate chunks keep PE busy through softmax
            if t + 1 < t_steps:
                gates1(t + 1)

            exp_tile(P_EB, wm_B, wtB, ws[:, 2:3], ws[:, 3:4])
            if wtA > 512:
                nc.vector.scalar_tensor_tensor(
                    out=wsA, in0=ws[:, 0:1], scalar=1.0, in1=ws[:, 1:2],
                    op0=OP.mult, op1=OP.add,
                )
                nc.vector.reciprocal(rinvA, wsA)
            else:
                nc.vector.reciprocal(rinvA, ws[:, 0:1])
            if wtB > 512:
                nc.vector.scalar_tensor_tensor(
                    out=wsB, in0=ws[:, 2:3], scalar=1.0, in1=ws[:, 3:4],
                    op0=OP.mult, op1=OP.add,
                )
                nc.vector.reciprocal(rinvB, wsB)
            else:
                nc.vector.reciprocal(rinvB, ws[:, 2:3])

            # ---- w^T, ctx, normalize, ctx^T ----
            wT_chunks(wm_A, wtA, wT_A, range(0, 4))
            wT_chunks(wm_B, wtB, wT_B, range(4, 8))

            ctx_mms(t, P_EA, range(0, 4), wT_A)
            nc.vector.tensor_scalar_mul(ctxs_A, P_EA[0:97, 0, 0:512], rinvA)
            ctx_mms(t, P_EB, range(4, 8), wT_B)
            nc.vector.tensor_scalar_mul(ctxs_B, P_EB[0:97, 0, 0:512], rinvB)

            for ck in range(4):
                nc.tensor.transpose(
                    P_CA[:, ck, 0:97],
                    ctxs_A[:, ck * 128 : (ck + 1) * 128],
                    id128b[0:97, 0:97],
                )
            nc.vector.tensor_copy(
                ctxT_all[:, :, (t + 1) * BL : (t + 1) * BL + 4],
                P_CA[:, :, 0:97:32],
            )
            for ck in range(4):
                nc.tensor.transpose(
                    P_CB[:, ck, 0:97],
                    ctxs_B[:, ck * 128 : (ck + 1) * 128],
                    id128b[0:97, 0:97],
                )
            nc.vector.tensor_copy(
                ctxT_all[:, :, (t + 1) * BL + 4 : (t + 2) * BL],
                P_CB[:, :, 0:97:32],
            )
            if t + 1 < t_steps:
                for ck in range(4):
                    gates2_k(t + 1, ck)


        # ---- leftover MLP units after the loop ----
        while mlp_units:
            emit_mlp_unit(mlp_units.pop(0))

    return nc


def assign_slots(out_lens):
    """Cross-core sort: global rank r -> (core r%8, slot r//8).
    Returns (perm [slot, core] -> original example idx, widths per slot)."""
    order = np.argsort(np.asarray(out_lens), kind="stable")
    perm = order.reshape(BL, NCORES)  # [slot, core]
    lens = np.asarray(out_lens)[perm]  # [slot, core]
    widths = tuple(slot_width(int(lens[m].max())) for m in range(BL))
    return perm, widths


def prep_core_inputs(core, perm, tokens, key_enc, value_enc, out_lens, t_steps=T):
    idx = perm[:, core]  # original example index per slot
    ke = key_enc[idx]  # [BL, S, H]
    kts = ke.transpose(0, 2, 1).reshape(BL, 4, 128, S).astype(BF16)

    vc = value_enc[idx]  # [BL, S, C]
    v = vc[:, :768].reshape(BL, 6, 128, C).astype(BF16)
    v6 = np.zeros((128, 2, C), np.float32)
    for m in range(BL):
        v6[32 * (m % 4) : 32 * (m % 4) + 32, m // 4] = vc[m, 768:800]

    oh = np.zeros((V, t_steps, BL), np.float32)
    tok = tokens[idx, :t_steps]  # [BL, t]
    for m in range(BL):
        oh[tok[m], np.arange(t_steps), m] = 1.0

    # energy bias rows: [slot-in-tile, (tileA h0, h1, tileB h0, h1), 512]
    eb = np.zeros((4, 4, 512), np.float32)
    lens = np.asarray(out_lens)[idx]
    s_idx = np.arange(S)
    for m in range(BL):
        bias_row = np.where(s_idx < lens[m], 0.0, NEG_BIAS)  # [S]
        ti, g = m // 4, m % 4
        eb[g, 2 * ti + 0, :] = bias_row[:512]
        eb[g, 2 * ti + 1, 0:288] = bias_row[512:]
    sel = np.zeros((4, 97), np.float32)
    for g in range(4):
        sel[g, 32 * g] = 1.0
    return {
        "kts": np.ascontiguousarray(kts),
        "v": np.ascontiguousarray(v),
        "v6": np.ascontiguousarray(v6.astype(BF16)),
        "oh": np.ascontiguousarray(oh.astype(BF16)),
        "ebias4": np.ascontiguousarray(eb.astype(BF16)),
        "sel4": np.ascontiguousarray(sel.astype(BF16)),
    }


def prep_shared_inputs(emb, W_ih, W_hh, b_ih, b_hh, W1, b1, W2, b2):
    # gate blocks reordered (i, f, o, g); i/f/o rows scaled by 0.5 for the
    # tanh-based sigmoid; h-feature columns scaled by 0.5 since h2 = 2h.
    perm = np.r_[0:1024, 1536:2048, 1024:1536]
    row_scale = np.ones((4 * H, 1), np.float64)
    row_scale[: 3 * H] = 0.5  # i, f, o rows (after perm)
    ew = ((emb @ W_ih[:, :E].T + b_ih + b_hh)[:, perm]) * row_scale[:, 0]
    wc = np.concatenate([W_ih[:, E:], W_hh * 0.5], axis=1)[perm] * row_scale
    wt = wc.T.reshape(8, 128, 4 * H).astype(BF16)
    W1s = W1.copy()
    W1s[:, :H] *= 0.5  # h columns (h2 = 2h)
    return {
        "ewt": np.ascontiguousarray(ew.reshape(V, 4, 512).astype(BF16)),
        "wt": np.ascontiguousarray(wt),
        "w1t": np.ascontiguousarray(W1s.T.reshape(8, 128, C).astype(BF16)),
        "w2t": np.ascontiguousarray(W2.T.reshape(4, 128, V).astype(BF16)),
        "b1c": np.ascontiguousarray(b1.reshape(4, 128).T.astype(np.float32)),
        "b2c": np.ascontiguousarray(b2[:, None].astype(np.float32)),
    }


_CACHE = {}


def _get_nc(t_steps, widths):
    key = (t_steps, widths)
    if key not in _CACHE:
        nc = build_bass(t_steps, widths)
        nc.finalize()
        _CACHE[key] = nc
    return _CACHE[key]


def _build_in_maps(t_steps, inputs):
    args = {k: np.asarray(v) for k, v in inputs.items()}
    tokens = args["tokens"].astype(np.int64)
    perm, widths = assign_slots(args["out_lens"])
    shared = prep_shared_inputs(
        args["emb"], args["W_ih"], args["W_hh"], args["b_ih"], args["b_hh"],
        args["W1"], args["b1"], args["W2"], args["b2"],
    )
    in_maps = []
    for core in range(NCORES):
        m = prep_core_inputs(
            core, perm, tokens, args["key_enc"], args["value_enc"],
            args["out_lens"], t_steps=t_steps,
        )
        m.update(shared)
        in_maps.append(m)
    return in_maps, perm, widths


def _unpack_out(outs, perm, t_steps):
    # outs: per-core [V, t_steps*BL] -> [B, t_steps, V] (unpermuted)
    full = np.empty((B, t_steps, V), np.float32)
    for core, o in enumerate(outs):
        r = o.reshape(V, t_steps, BL).transpose(2, 1, 0)  # [slot, t, V]
        full[perm[:, core]] = r
    return full


def run(t_steps=T, trace=False, **inputs):
    from concourse.bass_utils import run_bass_kernel_spmd

    in_maps, perm, widths = _build_in_maps(t_steps, inputs)
    nc = _get_nc(t_steps, widths)
    res = run_bass_kernel_spmd(nc, in_maps, list(range(NCORES)), trace=trace)
    outs = [np.asarray(r["out"], np.float32) for r in res.results]
    return _unpack_out(outs, perm, t_steps), res


def kernel(**inputs) -> np.ndarray:
    full, _ = run(t_steps=T, trace=False, **inputs)
    return full


def warm_timing(t_steps=T, n_iters=12, **inputs):
    """Time warm NEFF executions (device-resident inputs) as an HW-time proxy."""
    import time

    import jax
    from jax.sharding import Mesh, PartitionSpec
    from jax.experimental.shard_map import shard_map

    from concourse import bass2jax
    from concourse import mybir as _mybir
    from concourse.bass2jax import _bass_exec_p, install_neuronx_cc_hook

    install_neuronx_cc_hook()
    in_maps, perm, widths = _build_in_maps(t_steps, inputs)
    nc = _get_nc(t_steps, widths)

    partition_name = nc.partition_id_tensor.name if nc.partition_id_tensor else None
    in_names, out_names, out_avals, zero_outs = [], [], [], []
    for alloc in nc.m.functions[0].allocations:
        if not isinstance(alloc, _mybir.MemoryLocationSet):
            continue
        name = alloc.memorylocations[0].name
        if alloc.kind == "ExternalInput":
            if name != partition_name:
                in_names.append(name)
        elif alloc.kind == "ExternalOutput":
            out_names.append(name)
            shape = tuple(alloc.tensor_shape)
            dtype = _mybir.dt.np(alloc.dtype)
            out_avals.append(jax.core.ShapedArray(shape, dtype))
            zero_outs.append(np.zeros(shape, dtype))
    n_params = len(in_names)
    n_outs = len(out_avals)
    in_names.extend(out_names)
    if partition_name:
        in_names.append(partition_name)

    def _body(*a):
        operands = list(a)
        if partition_name:
            operands.append(bass2jax.partition_id_tensor())
        return tuple(
            _bass_exec_p.bind(
                *operands,
                out_avals=tuple(out_avals),
                in_names=tuple(in_names),
                out_names=tuple(out_names),
                lowering_input_output_aliases=(),
                sim_require_finite=True,
                sim_require_nnan=True,
                nc=nc,
            )
        )

    devices = jax.devices()[:NCORES]
    mesh = Mesh(np.asarray(devices), ("core",))
    sharded = jax.jit(
        shard_map(
            _body,
            mesh=mesh,
            in_specs=(PartitionSpec("core"),) * (n_params + n_outs),
            out_specs=(PartitionSpec("core"),) * len(out_names),
            check_rep=False,
        ),
        keep_unused=True,
    )
    per_core = [[np.asarray(m[nm]) for nm in in_names[:n_params]] for m in in_maps]
    concat_in = [
        jax.device_put(np.concatenate([per_core[c][i] for c in range(NCORES)], axis=0))
        for i in range(n_params)
    ]
    concat_zeros = [
        jax.device_put(np.zeros((NCORES * z.shape[0], *z.shape[1:]), z.dtype))
        for z in zero_outs
    ]
    outs = sharded(*concat_in, *concat_zeros)
    jax.block_until_ready(outs)
    best = None
    for _ in range(n_iters):
        t0 = time.time()
        outs = sharded(*concat_in, *concat_zeros)
        jax.block_until_ready(outs)
        dt = time.time() - t0
        best = dt if best is None else min(best, dt)

    oarr = np.asarray(outs[out_names.index("out")]).reshape(NCORES, V, t_steps * BL)
    full = _unpack_out(list(oarr), perm, t_steps)
    return best, full
